# revision 1
# baseline (speedup 1.0000x reference)
"""Trainium2 Bass kernel for nn_DeepConv1d (self-contained).

Math (per batch b):
  xr   = linear-interp(deep, 1024 -> 4096)           # commutes with 1x1 conv
  y    = conv_w @ xr + conv_b                        # == interp(conv_w @ deep + conv_b)
  xs   = GAMA*(y-mean)/(var_unbiased+EPS)            # per-channel over n
  loss_k[c,l] = sech^2(xs_pad[c,l+k]-xs_pad[c,l+3])  # k=0..6, reflect pad 3
  S    = sum_k loss_k ;  W_k = (loss_k/S)*x_pad[:,l+k]
  out[o,l] = sum_{c,k} fc_w[o, 7c+k] * W_k[c,l]

On-chip identities:
  - interp(conv(.)) == conv(interp(.)); interp via first differences D.
  - sech^2(d) = 4*sigmoid(2d)*sigmoid(-2d); the normalization scale
    f = GAMA/(var+EPS) folds into the sigmoid's per-partition scale
    (the mean cancels inside differences). The *4 is folded into the
    reciprocal cast (G4 = 4/S) and fc[:,3] host-scaled by 1/4.
  - loss_k arrays are shifted views of 3 gap arrays lv_g (g = |k-3|):
      k<3: loss_k[l] = lv_g[l+k] (g=3-k);  k>3: loss_k[l] = lv_g[l+3] (g=k-3).

Layout: 2 batches per core packed on 128 partitions (64 channels each).
Post-sigmoid pipeline is chunked into 4 l-chunks of 1024 so DVE/GPSIMD
products, ACT casts, PE matmuls and output DMAs pipeline against each other.
"""
import contextlib

import numpy as np
import ml_dtypes

import concourse.bass as bass
import concourse.bacc as bacc_mod
import concourse.mybir as mybir
import concourse.tile as tile
from concourse.bass_utils import run_bass_kernel_spmd

bf16 = ml_dtypes.bfloat16
AF = mybir.ActivationFunctionType
ALU = mybir.AluOpType

KS = 7
PAD = 3
GAMA = 0.5
EPS = 1e-9
N = 4096
ND = 1024
NP = N + 2 * PAD       # 4102
L3 = N + PAD           # 4099: lv array length
NCORES = 8
NCH = 4                # l-chunks
CW = N // NCH          # 1024

F32 = mybir.dt.float32
BF = mybir.dt.bfloat16

# Fraction of mult-class elementwise ops done on DVE (rest on GPSIMD).
FRAC = {"lv": 0.62, "P": 0.62, "GL": 0.62}


def _even(v):
    return int(v) & ~1


def kernel_body(tc, xp_d, cwdp_d, cb_d, fck_d, out_d):
    nc = tc.nc

    def smul(key, out, a, b, width):
        """a*b elementwise, split DVE/GPSIMD at an even boundary."""
        c = _even(width * FRAC[key])
        if c > 0:
            nc.vector.tensor_mul(out=out[:, 0:c], in0=a[:, 0:c], in1=b[:, 0:c])
        if c < width:
            nc.gpsimd.tensor_mul(out=out[:, c:width], in0=a[:, c:width],
                                 in1=b[:, c:width])

    ctx = contextlib.ExitStack()
    with ctx:
        io = ctx.enter_context(tc.tile_pool(name="io", bufs=1))
        mid = ctx.enter_context(tc.tile_pool(name="mid", bufs=1))
        loss = ctx.enter_context(tc.tile_pool(name="loss", bufs=1))
        ck = ctx.enter_context(tc.tile_pool(name="ck", bufs=2))
        stp = ctx.enter_context(tc.tile_pool(name="stp", bufs=3))
        pp = ctx.enter_context(tc.tile_pool(name="pp", bufs=1, space="PSUM"))
        ppa = ctx.enter_context(tc.tile_pool(name="ppa", bufs=4, space="PSUM"))

        # ---------------- input DMAs (small first) ----------------
        cwdp = io.tile([32, 128 + ND], F32, tag="cwdp")
        nc.sync.dma_start(out=cwdp, in_=cwdp_d[:, :])
        cb = io.tile([128, 1], F32, tag="cb")
        nc.sync.dma_start(out=cb, in_=cb_d[:, :])
        fck = io.tile([128, KS, 128], BF, tag="fck")
        nc.sync.dma_start(out=fck, in_=fck_d[:, :, :])
        xp = io.tile([128, NP], BF, tag="xp")          # x reflect-padded
        xs1 = io.tile([128, NP - 1], BF, tag="xs1")    # same, shifted 1 elem
        nc.sync.dma_start(out=xp, in_=xp_d[:, :])
        nc.sync.dma_start(out=xs1, in_=xp_d[:, 1:NP])
        cw = cwdp[:, 0:128]
        dp = cwdp[:, 128:128 + ND]
        warm = mid.tile([128, 1], F32, tag="warm")
        nc.scalar.activation(out=warm, in_=cb, func=AF.Sigmoid, scale=1.0)

        # ---------------- conv (PE) + bias (ACT) ----------------
        ys_ps = pp.tile([128, ND], F32, tag="ys")
        for h in range(2):
            nc.tensor.matmul(
                out=ys_ps[:, h * 512:(h + 1) * 512],
                lhsT=cw,
                rhs=dp[:, h * 512:(h + 1) * 512],
                start=True, stop=True,
            )
        ys = mid.tile([128, ND], F32, tag="ys_sb")
        nc.scalar.activation(out=ys, in_=ys_ps, func=AF.Identity, bias=cb, scale=1.0)

        # ---------------- interp -> ypad (bf16), with free sum(y) ----------
        Dp = mid.tile([128, ND + 1], BF, tag="Dp")
        nc.vector.memset(Dp[:, 0:1], 0.0)
        nc.vector.memset(Dp[:, ND:ND + 1], 0.0)
        nc.vector.tensor_sub(out=Dp[:, 1:ND], in0=ys[:, 1:ND], in1=ys[:, 0:ND - 1])
        # interp via pair-interleaved ops: y[4j+{0,1}] = ys[j] - {3,1}/8*D[j],
        # y[4j+{2,3}] = ys[j] + {1,3}/8*D[j+1]. Pair writes are 4B-contiguous,
        # avoiding the 16B-stride write penalty of per-phase ops.
        c12 = mid.tile([128, 2], F32, tag="c12")
        nc.vector.memset(c12[:, 0:1], 0.375)
        nc.vector.memset(c12[:, 1:2], 0.125)
        c21 = mid.tile([128, 2], F32, tag="c21")
        nc.vector.memset(c21[:, 0:1], 0.125)
        nc.vector.memset(c21[:, 1:2], 0.375)

        def _rep2(ap_, off):
            return bass.AP(tensor=ap_.tensor, offset=ap_.offset + off,
                           ap=[list(ap_.ap[0]), [1, ND], [0, 2]])

        def _bcast(ap_):
            return bass.AP(tensor=ap_.tensor, offset=ap_.offset,
                           ap=[list(ap_.ap[0]), [0, ND], [1, 2]])

        DD12 = mid.tile([128, ND, 2], BF, tag="DD12")
        nc.vector.tensor_mul(out=DD12, in0=_rep2(Dp, 0), in1=_bcast(c12))
        DD21 = mid.tile([128, ND, 2], BF, tag="DD21")
        nc.vector.tensor_mul(out=DD21, in0=_rep2(Dp, 1), in1=_bcast(c21))

        ypad = mid.tile([128, NP], BF, tag="ypad")
        y4 = ypad[:, PAD:PAD + N].rearrange("p (j r) -> p j r", r=4)
        nc.vector.tensor_sub(out=y4[:, :, 0:2], in0=_rep2(ys, 0), in1=DD12)
        nc.gpsimd.tensor_add(out=y4[:, :, 2:4], in0=_rep2(ys, 0), in1=DD21)
        # sum(y) == 4*sum(ys) exactly: the D-term contributions telescope to
        # D[ND]-D[0] = 0 (both ends are zero by construction).
        sumys = mid.tile([128, 1], F32, tag="sumys")
        nc.vector.tensor_reduce(
            out=sumys, in_=ys, axis=mybir.AxisListType.X, op=ALU.add)
        # reflect edges: ypad[2-i] = ypad[4+i], ypad[N+3+i] = ypad[N+1-i]
        for i in range(3):
            nc.vector.tensor_copy(out=ypad[:, 2 - i:3 - i], in_=ypad[:, 4 + i:5 + i])
            nc.vector.tensor_copy(
                out=ypad[:, N + 3 + i:N + 4 + i], in_=ypad[:, N + 1 - i:N + 2 - i])

        # ---------------- stats -> sigmoid scales ----------------
        y_main = ypad[:, PAD:PAD + N]
        sq = [mid.tile([128, 1], F32, tag=f"sq{i}", name=f"sq{i}")
              for i in range(4)]
        for i in range(4):
            dump = pp.tile([128, ND], F32, tag="ys", name=f"dump{i}")
            nc.scalar.activation(out=dump, in_=y_main[:, i * ND:(i + 1) * ND],
                                 func=AF.Square, accum_out=sq[i])
        sum_y = mid.tile([128, 1], F32, tag="sum_y")
        nc.vector.tensor_scalar_mul(out=sum_y, in0=sumys, scalar1=4.0)
        tc_ = mid.tile([128, 1], F32, tag="tc_")
        nc.vector.tensor_add(out=tc_, in0=sq[0], in1=sq[1])
        td = mid.tile([128, 1], F32, tag="td")
        nc.vector.tensor_add(out=td, in0=sq[2], in1=sq[3])
        sum_y2 = mid.tile([128, 1], F32, tag="sum_y2")
        nc.vector.tensor_add(out=sum_y2, in0=tc_, in1=td)
        # mean = sum_y/N; var = (sum_y2 - sum_y*mean)/(N-1); f = GAMA/(var+EPS)
        mean = mid.tile([128, 1], F32, tag="mean")
        nc.vector.tensor_scalar_mul(out=mean, in0=sum_y, scalar1=1.0 / N)
        t0 = mid.tile([128, 1], F32, tag="t0")
        nc.vector.tensor_mul(out=t0, in0=sum_y, in1=mean)
        t2 = mid.tile([128, 1], F32, tag="t2")
        nc.vector.tensor_sub(out=t2, in0=sum_y2, in1=t0)
        denom = mid.tile([128, 1], F32, tag="denom")
        nc.vector.tensor_scalar(out=denom, in0=t2, scalar1=1.0 / (N - 1),
                                scalar2=EPS, op0=ALU.mult, op1=ALU.add)
        inv = mid.tile([128, 1], F32, tag="inv")
        nc.vector.reciprocal(out=inv, in_=denom)
        f2p = mid.tile([128, 1], F32, tag="f2p")
        f2n = mid.tile([128, 1], F32, tag="f2n")
        nc.vector.tensor_scalar_mul(out=f2p, in0=inv, scalar1=2.0 * GAMA)
        nc.vector.tensor_scalar_mul(out=f2n, in0=inv, scalar1=-2.0 * GAMA)

        # ---------------- gap diffs (bf16, DVE 2x adds) ----------------
        dy1 = loss.tile([128, L3], BF, tag="T1")
        dy2b = loss.tile([128, L3], BF, tag="T2")
        dy3 = loss.tile([128, L3], BF, tag="T3")
        nc.vector.tensor_sub(out=dy1, in0=ypad[:, 1:1 + L3], in1=ypad[:, 0:L3])
        nc.vector.tensor_sub(out=dy2b, in0=ypad[:, 3:3 + L3], in1=ypad[:, 1:1 + L3])
        nc.vector.tensor_sub(out=dy3, in0=ypad[:, 3:3 + L3], in1=ypad[:, 0:L3])

        # ---------------- sigmoids (ACT) + lv' = sa*sb ----------------
        # sa in fp32; lv'neg = (sa-1)*sa = -sigmoid'(z) computed in one DVE
        # STT pass (fp32 internal -> no cancellation). All downstream lv users
        # see a negated value; compensated by host-negating fc for k != 3.
        sa3 = loss.tile([128, L3], F32, tag="S4")
        sa2 = loss.tile([128, L3], F32, tag="S2")
        sa1 = loss.tile([128, L3], F32, tag="S0")
        nc.scalar.activation(out=sa3, in_=dy3, func=AF.Sigmoid, scale=f2p)
        nc.scalar.activation(out=sa2, in_=dy2b, func=AF.Sigmoid, scale=f2p)
        nc.scalar.activation(out=sa1, in_=dy1, func=AF.Sigmoid, scale=f2p)
        lv3 = loss.tile([128, L3], BF, tag="T3")
        lv2b = loss.tile([128, L3], BF, tag="T2")
        lv1 = loss.tile([128, L3], BF, tag="T1")
        nc.vector.scalar_tensor_tensor(
            out=lv3, in0=sa3, scalar=1.0, in1=sa3, op0=ALU.subtract, op1=ALU.mult)
        nc.vector.scalar_tensor_tensor(
            out=lv2b, in0=sa2, scalar=1.0, in1=sa2, op0=ALU.subtract, op1=ALU.mult)
        nc.vector.scalar_tensor_tensor(
            out=lv1, in0=sa1, scalar=1.0, in1=sa1, op0=ALU.subtract, op1=ALU.mult)

        # ---------------- S partials (DVE 2x adds) + P products ----------
        m1c = loss.tile([128, N], BF, tag="S1")
        m2 = loss.tile([128, N], BF, tag="S3")
        m3 = loss.tile([128, N], BF, tag="S5")
        nc.vector.tensor_add(out=m1c, in0=lv1[:, 2:N + 2], in1=lv1[:, 3:N + 3])
        nc.vector.tensor_add(out=m2, in0=lv2b[:, 0:N], in1=lv2b[:, 2:N + 2])
        nc.vector.tensor_add(out=m3, in0=lv3[:, 0:N], in1=lv3[:, 3:N + 3])
        s12 = loss.tile([128, N], BF, tag="M1")
        nc.vector.tensor_add(out=s12, in0=m3, in1=m2)
        msum = loss.tile([128, N], BF, tag="S5")   # after m3 consumed
        nc.vector.tensor_add(out=msum, in0=s12, in1=m1c)

        P12 = loss.tile([128, N], BF, tag="S0")
        P21 = loss.tile([128, N], BF, tag="S2")
        P30 = loss.tile([128, N], BF, tag="S4")
        smul("P", P12, lv1[:, 2:N + 2], xp[:, 2:N + 2], N)
        smul("P", P21, lv2b[:, 0:N], xs1[:, 0:N], N)
        smul("P", P30, lv3[:, 0:N], xp[:, 0:N], N)

        # ---------------- chunked: S32 -> G4 -> GL/W -> GEMM -> out ------
        for c in range(NCH):
            lo = c * CW
            S32 = ck.tile([128, CW], F32, tag="S32", name=f"S32_{c}")
            nc.vector.tensor_scalar(out=S32, in0=msum[:, lo:lo + CW],
                                    scalar1=-4.0, scalar2=1.0,
                                    op0=ALU.mult, op1=ALU.add)
            G32 = ck.tile([128, CW], F32, tag="S32", name=f"G32_{c}")
            nc.vector.reciprocal_approx_fast(out=G32, in_=S32)
            G4 = ck.tile([128, CW], BF, tag="G4", name=f"G4_{c}")
            nc.vector.tensor_scalar_mul(out=G4, in0=G32, scalar1=4.0)

            GL1 = ck.tile([128, CW], BF, tag="GL1", name=f"GL1_{c}")
            GL2 = ck.tile([128, CW], BF, tag="GL2", name=f"GL2_{c}")
            GL3 = ck.tile([128, CW], BF, tag="GL3", name=f"GL3_{c}")
            smul("GL", GL1, lv1[:, lo + 3:lo + 3 + CW], G4, CW)
            smul("GL", GL2, lv2b[:, lo + 2:lo + 2 + CW], G4, CW)
            smul("GL", GL3, lv3[:, lo + 3:lo + 3 + CW], G4, CW)

            W = [ck.tile([128, CW], BF, tag=f"W{k}", name=f"W{k}_{c}")
                 for k in range(KS)]
            nc.vector.tensor_mul(out=W[0], in0=G4, in1=P30[:, lo:lo + CW])
            nc.vector.tensor_mul(out=W[1], in0=G4, in1=P21[:, lo:lo + CW])
            nc.vector.tensor_mul(out=W[2], in0=G4, in1=P12[:, lo:lo + CW])
            nc.vector.tensor_mul(out=W[3], in0=G4, in1=xs1[:, lo + 2:lo + 2 + CW])
            nc.vector.tensor_mul(out=W[4], in0=GL1, in1=xp[:, lo + 4:lo + 4 + CW])
            nc.gpsimd.tensor_mul(out=W[5], in0=GL2, in1=xs1[:, lo + 4:lo + 4 + CW])
            nc.gpsimd.tensor_mul(out=W[6], in0=GL3, in1=xp[:, lo + 6:lo + 6 + CW])

            for b in range(2):
                prow = slice(64 * b, 64 * (b + 1))
                for sub in range(CW // 512):
                    acc = ppa.tile([128, 512], F32, tag="acc",
                                   name=f"acc_{c}_{b}_{sub}")
                    cs = slice(sub * 512, (sub + 1) * 512)
                    for k in range(KS):
                        nc.tensor.matmul(
                            out=acc,
                            lhsT=fck[prow, k, :],
                            rhs=W[k][prow, cs],
                            start=(k == 0), stop=(k == KS - 1),
                        )
                    stage = stp.tile([128, 512], F32, tag="stage",
                                     name=f"stage_{c}_{b}_{sub}")
                    if (b + sub) % 2 == 0:
                        nc.scalar.copy(out=stage, in_=acc)
                    else:
                        nc.vector.tensor_copy(out=stage, in_=acc)
                    nc.sync.dma_start(
                        out=out_d[:, b, lo + sub * 512:lo + (sub + 1) * 512],
                        in_=stage)


def build_nc():
    nc = bacc_mod.Bacc(None, target_bir_lowering=False)
    xp_d = nc.dram_tensor("xp", [128, NP], BF, kind="ExternalInput")
    cwdp_d = nc.dram_tensor("cwdp", [32, 128 + ND], F32, kind="ExternalInput")
    cb_d = nc.dram_tensor("cb", [128, 1], F32, kind="ExternalInput")
    fck_d = nc.dram_tensor("fck", [128, KS, 128], BF, kind="ExternalInput")
    out_d = nc.dram_tensor("out", [128, 2, N], F32, kind="ExternalOutput")
    with tile.TileContext(nc) as tc:
        kernel_body(tc, xp_d, cwdp_d, cb_d, fck_d, out_d)
    nc.compile()
    return nc


def prep_inputs(deep, x, conv_w, conv_b, fc_w):
    deep = np.asarray(deep, np.float32)
    x = np.asarray(x, np.float32)
    conv_w = np.asarray(conv_w, np.float32)
    conv_b = np.asarray(conv_b, np.float32)
    fc_w = np.asarray(fc_w, np.float32)

    xpad = np.pad(x, ((0, 0), (0, 0), (PAD, PAD)), mode="reflect")
    xp_all = np.ascontiguousarray(xpad.reshape(NCORES, 128, NP)).astype(bf16)
    dp_all = np.ascontiguousarray(deep.reshape(NCORES, 32, ND))
    cw_blk = np.zeros((32, 128), np.float32)
    cw_blk[0:16, 0:64] = conv_w.T
    cw_blk[16:32, 64:128] = conv_w.T
    cb = np.ascontiguousarray(
        np.concatenate([conv_b, conv_b]).reshape(128, 1).astype(np.float32))
    fc3 = fc_w.reshape(128, 64, KS)
    fck_half = np.transpose(fc3, (1, 2, 0)).copy()
    fck_half *= -1.0              # lv' is computed negated on-chip
    fck_half[:, PAD, :] *= -0.25  # W_3 = G4*x = 4*(G*x), not lv-scaled
    fck = np.ascontiguousarray(
        np.concatenate([fck_half, fck_half], axis=0)).astype(bf16)
    return [
        {"xp": np.ascontiguousarray(xp_all[ci]),
         "cwdp": np.ascontiguousarray(
             np.concatenate([cw_blk, dp_all[ci]], axis=1)),
         "cb": cb, "fck": fck}
        for ci in range(NCORES)
    ]


def gather_out(results):
    out_full = np.empty((16, 128, N), np.float32)
    for ci in range(NCORES):
        o = results[ci]["out"]
        out_full[2 * ci] = o[:, 0]
        out_full[2 * ci + 1] = o[:, 1]
    return out_full


_CACHED = {}


def _get_nc():
    if "nc" not in _CACHED:
        _CACHED["nc"] = build_nc()
    return _CACHED["nc"]


def kernel(deep, x, conv_w, conv_b, fc_w):
    in_maps = prep_inputs(deep, x, conv_w, conv_b, fc_w)
    nc = _get_nc()
    res = run_bass_kernel_spmd(nc, in_maps, core_ids=list(range(NCORES)))
    return gather_out(res.results)



# revision 3
# speedup vs baseline: 1.1696x; 1.1696x over previous
"""Trainium2 Bass kernel for nn_DeepConv1d (self-contained).

Math (per batch b):
  xr   = linear-interp(deep, 1024 -> 4096)           # commutes with 1x1 conv
  y    = conv_w @ xr + conv_b                        # == interp(conv_w @ deep + conv_b)
  xs   = GAMA*(y-mean)/(var_unbiased+EPS)            # per-channel over n
  loss_k[c,l] = sech^2(xs_pad[c,l+k]-xs_pad[c,l+3])  # k=0..6, reflect pad 3
  S    = sum_k loss_k ;  W_k = (loss_k/S)*x_pad[:,l+k]
  out[o,l] = sum_{c,k} fc_w[o, 7c+k] * W_k[c,l]

On-chip identities:
  - interp(conv(.)) == conv(interp(.)); interp via first differences D.
  - sech^2(d) = 4*sigmoid(2d)*sigmoid(-2d); the normalization scale
    f = GAMA/(var+EPS) folds into the sigmoid's per-partition scale.
  - lv = (sa-1)*sa = -sigmoid'(z), computed in bf16 (2x DVE mode).
    Downstream sees negated lv; compensated by host-negating fc (k != 3).
  - S/4 = 0.25 + sum sigmoid' accumulated ON THE PE: 7 matmuls with a
    negated-identity lhsT (6 shifted lv views + a -0.25 const tile) into
    PSUM, then one DVE reciprocal_approx_fast gives G4 = 4/S directly.
  - loss_k arrays are shifted views of 3 gap arrays lv_g (g = |k-3|).

Layout: 2 batches per core packed on 128 partitions (64 channels each).
Work distribution: DVE does bf16 2x elementwise; ACT does sigmoids,
squares, bias, G4 casts and most PSUM->SBUF stage copies; PE does conv,
msum accumulation and the final GEMM (k-outer so lhsT is reused across
4 PSUM banks); GPSIMD only touches private copies of xp/xs1 (W5/W6) and
a few stage copies, to avoid SBUF contention with DVE.
Gap arrays are processed in 2 column halves so chunk 0's GEMM starts
while the second halves are still in the sigmoid stage.
"""
import contextlib

import numpy as np
import ml_dtypes

import concourse.bass as bass
import concourse.bacc as bacc_mod
import concourse.mybir as mybir
import concourse.tile as tile
from concourse.bass_utils import run_bass_kernel_spmd

bf16 = ml_dtypes.bfloat16
AF = mybir.ActivationFunctionType
ALU = mybir.AluOpType

KS = 7
PAD = 3
GAMA = 0.5
EPS = 1e-9
N = 4096
ND = 1024
NP = N + 2 * PAD       # 4102
L3 = N + PAD           # 4099: lv array length
NCORES = 8
NCH = 4                # l-chunks
CW = N // NCH          # 1024
HA = 2048              # first-half width of gap arrays

F32 = mybir.dt.float32
BF = mybir.dt.bfloat16


def kernel_body(tc, xp_d, cwdp_d, cb_d, fck_d, eyen_d, out_d):
    nc = tc.nc

    ctx = contextlib.ExitStack()
    with ctx:
        io = ctx.enter_context(tc.tile_pool(name="io", bufs=1))
        mid = ctx.enter_context(tc.tile_pool(name="mid", bufs=1))
        loss = ctx.enter_context(tc.tile_pool(name="loss", bufs=1))
        ck = ctx.enter_context(tc.tile_pool(name="ck", bufs=2))
        stp = ctx.enter_context(tc.tile_pool(name="stp", bufs=4))
        pp = ctx.enter_context(tc.tile_pool(name="pp", bufs=1, space="PSUM"))
        msq = ctx.enter_context(tc.tile_pool(name="msq", bufs=2, space="PSUM"))
        ppa = ctx.enter_context(tc.tile_pool(name="ppa", bufs=4, space="PSUM"))

        # ---------------- input DMAs (small first) ----------------
        cwdp = io.tile([32, 128 + ND], F32, tag="cwdp")
        nc.sync.dma_start(out=cwdp, in_=cwdp_d[:, :])
        cb = io.tile([128, 1], F32, tag="cb")
        nc.sync.dma_start(out=cb, in_=cb_d[:, :])
        fck = io.tile([128, KS, 128], BF, tag="fck")
        nc.sync.dma_start(out=fck, in_=fck_d[:, :, :])
        eyen = io.tile([128, 128], BF, tag="eyen")
        nc.sync.dma_start(out=eyen, in_=eyen_d[:, :])
        xp = io.tile([128, NP], BF, tag="xp")          # x reflect-padded
        xs1 = io.tile([128, NP - 1], BF, tag="xs1")    # same, shifted 1 elem
        nc.sync.dma_start(out=xp, in_=xp_d[:, :])
        nc.sync.dma_start(out=xs1, in_=xp_d[:, 1:NP])
        # private copies for GPSIMD (avoid SBUF stream contention with DVE)
        xpg = io.tile([128, NP], BF, tag="xpg")
        xs1g = io.tile([128, NP - 1], BF, tag="xs1g")
        nc.sync.dma_start(out=xpg, in_=xp_d[:, :])
        nc.sync.dma_start(out=xs1g, in_=xp_d[:, 1:NP])

        warm = mid.tile([128, 1], F32, tag="warm")
        nc.scalar.activation(out=warm, in_=cb, func=AF.Sigmoid, scale=1.0)
        negq = mid.tile([128, 512], BF, tag="negq")
        nc.vector.memset(negq, -0.25)

        cw = cwdp[:, 0:128]
        dp = cwdp[:, 128:128 + ND]

        # ---------------- conv (PE) + bias (ACT, free sum(y)) ------------
        ys_ps = pp.tile([128, ND], F32, tag="ys")
        for h in range(2):
            nc.tensor.matmul(
                out=ys_ps[:, h * 512:(h + 1) * 512],
                lhsT=cw,
                rhs=dp[:, h * 512:(h + 1) * 512],
                start=True, stop=True,
            )
        ys = mid.tile([128, ND], F32, tag="ys_sb")
        sumys = mid.tile([128, 1], F32, tag="sumys")
        nc.scalar.activation(out=ys, in_=ys_ps, func=AF.Identity, bias=cb,
                             scale=1.0, accum_out=sumys)

        # ---------------- interp -> ypad (bf16) ----------
        Dp = mid.tile([128, ND + 1], BF, tag="Dp")
        nc.vector.memset(Dp[:, 0:1], 0.0)
        nc.vector.memset(Dp[:, ND:ND + 1], 0.0)
        nc.vector.tensor_sub(out=Dp[:, 1:ND], in0=ys[:, 1:ND], in1=ys[:, 0:ND - 1])
        # interp via pair-interleaved ops: y[4j+{0,1}] = ys[j] - {3,1}/8*D[j],
        # y[4j+{2,3}] = ys[j] + {1,3}/8*D[j+1].
        c12 = mid.tile([128, 2], F32, tag="c12")
        nc.vector.memset(c12[:, 0:1], 0.375)
        nc.vector.memset(c12[:, 1:2], 0.125)
        c21 = mid.tile([128, 2], F32, tag="c21")
        nc.vector.memset(c21[:, 0:1], 0.125)
        nc.vector.memset(c21[:, 1:2], 0.375)

        def _rep2(ap_, off):
            return bass.AP(tensor=ap_.tensor, offset=ap_.offset + off,
                           ap=[list(ap_.ap[0]), [1, ND], [0, 2]])

        def _bcast(ap_):
            return bass.AP(tensor=ap_.tensor, offset=ap_.offset,
                           ap=[list(ap_.ap[0]), [0, ND], [1, 2]])

        DD12 = mid.tile([128, ND, 2], BF, tag="DD12")
        nc.vector.tensor_mul(out=DD12, in0=_rep2(Dp, 0), in1=_bcast(c12))
        DD21 = mid.tile([128, ND, 2], BF, tag="DD21")
        nc.vector.tensor_mul(out=DD21, in0=_rep2(Dp, 1), in1=_bcast(c21))

        ypad = mid.tile([128, NP], BF, tag="ypad")
        y4 = ypad[:, PAD:PAD + N].rearrange("p (j r) -> p j r", r=4)
        nc.vector.tensor_sub(out=y4[:, :, 0:2], in0=_rep2(ys, 0), in1=DD12)
        nc.vector.tensor_add(out=y4[:, :, 2:4], in0=_rep2(ys, 0), in1=DD21)
        # reflect edges: ypad[2-i] = ypad[4+i], ypad[N+3+i] = ypad[N+1-i]
        for i in range(3):
            nc.vector.tensor_copy(out=ypad[:, 2 - i:3 - i], in_=ypad[:, 4 + i:5 + i])
            nc.vector.tensor_copy(
                out=ypad[:, N + 3 + i:N + 4 + i], in_=ypad[:, N + 1 - i:N + 2 - i])

        # ---------------- stats -> sigmoid scale ----------------
        y_main = ypad[:, PAD:PAD + N]
        sq = [mid.tile([128, 1], F32, tag=f"sq{i}", name=f"sq{i}")
              for i in range(4)]
        for i in range(4):
            dump = pp.tile([128, ND], F32, tag="ys", name=f"dump{i}")
            nc.scalar.activation(out=dump, in_=y_main[:, i * ND:(i + 1) * ND],
                                 func=AF.Square, accum_out=sq[i])
        # sum(y) == 4*sum(ys) exactly (D-term contributions telescope to 0)
        sum_y = mid.tile([128, 1], F32, tag="sum_y")
        nc.vector.tensor_scalar_mul(out=sum_y, in0=sumys, scalar1=4.0)
        tc_ = mid.tile([128, 1], F32, tag="tc_")
        nc.vector.tensor_add(out=tc_, in0=sq[0], in1=sq[1])
        td = mid.tile([128, 1], F32, tag="td")
        nc.vector.tensor_add(out=td, in0=sq[2], in1=sq[3])
        sum_y2 = mid.tile([128, 1], F32, tag="sum_y2")
        nc.vector.tensor_add(out=sum_y2, in0=tc_, in1=td)
        # mean = sum_y/N; var = (sum_y2 - sum_y*mean)/(N-1); f = GAMA/(var+EPS)
        mean = mid.tile([128, 1], F32, tag="mean")
        nc.vector.tensor_scalar_mul(out=mean, in0=sum_y, scalar1=1.0 / N)
        t0 = mid.tile([128, 1], F32, tag="t0")
        nc.vector.tensor_mul(out=t0, in0=sum_y, in1=mean)
        t2 = mid.tile([128, 1], F32, tag="t2")
        nc.vector.tensor_sub(out=t2, in0=sum_y2, in1=t0)
        denom = mid.tile([128, 1], F32, tag="denom")
        nc.vector.tensor_scalar(out=denom, in0=t2, scalar1=1.0 / (N - 1),
                                scalar2=EPS, op0=ALU.mult, op1=ALU.add)
        inv = mid.tile([128, 1], F32, tag="inv")
        nc.vector.reciprocal(out=inv, in_=denom)
        f2p = mid.tile([128, 1], F32, tag="f2p")
        nc.vector.tensor_scalar_mul(out=f2p, in0=inv, scalar1=2.0 * GAMA)

        # ---------------- gap diffs, sigmoids, lv, P (in halves) ---------
        # half ranges over the L3-long gap arrays
        halves = [(0, HA), (HA, L3)]
        dy1 = loss.tile([128, L3], BF, tag="T1")
        dy2b = loss.tile([128, L3], BF, tag="T2")
        dy3 = loss.tile([128, L3], BF, tag="T3")
        sa1 = loss.tile([128, L3], BF, tag="S1")
        sa2 = loss.tile([128, L3], BF, tag="S2")
        sa3 = loss.tile([128, L3], BF, tag="S3")
        lv1 = loss.tile([128, L3], BF, tag="T1")   # in-place over dy
        lv2b = loss.tile([128, L3], BF, tag="T2")
        lv3 = loss.tile([128, L3], BF, tag="T3")
        P12 = loss.tile([128, N], BF, tag="P0")
        P21 = loss.tile([128, N], BF, tag="P2")
        P30 = loss.tile([128, N], BF, tag="P4")

        for (a, b) in halves:
            w = b - a
            nc.vector.tensor_sub(out=dy3[:, a:b], in0=ypad[:, 3 + a:3 + b],
                                 in1=ypad[:, a:b])
            nc.vector.tensor_sub(out=dy2b[:, a:b], in0=ypad[:, 3 + a:3 + b],
                                 in1=ypad[:, 1 + a:1 + b])
            nc.vector.tensor_sub(out=dy1[:, a:b], in0=ypad[:, 1 + a:1 + b],
                                 in1=ypad[:, a:b])

        # sigmoids (ACT, bf16 out) ordered so chunk-0 deps resolve first
        for (a, b) in halves:
            nc.scalar.activation(out=sa3[:, a:b], in_=dy3[:, a:b],
                                 func=AF.Sigmoid, scale=f2p)
            nc.scalar.activation(out=sa2[:, a:b], in_=dy2b[:, a:b],
                                 func=AF.Sigmoid, scale=f2p)
            nc.scalar.activation(out=sa1[:, a:b], in_=dy1[:, a:b],
                                 func=AF.Sigmoid, scale=f2p)

        # lv = (sa-1)*sa = -sigmoid'  (bf16 2x STT); P = lv * x_shift
        # P split boundaries chosen so each half only needs its own lv half.
        for hi, (a, b) in enumerate(halves):
            nc.vector.scalar_tensor_tensor(
                out=lv3[:, a:b], in0=sa3[:, a:b], scalar=1.0, in1=sa3[:, a:b],
                op0=ALU.subtract, op1=ALU.mult)
            pa, pb = (0, HA) if hi == 0 else (HA, N)
            nc.vector.tensor_mul(out=P30[:, pa:pb], in0=lv3[:, pa:pb],
                                 in1=xp[:, pa:pb])
            nc.vector.scalar_tensor_tensor(
                out=lv2b[:, a:b], in0=sa2[:, a:b], scalar=1.0, in1=sa2[:, a:b],
                op0=ALU.subtract, op1=ALU.mult)
            nc.vector.tensor_mul(out=P21[:, pa:pb], in0=lv2b[:, pa:pb],
                                 in1=xs1[:, pa:pb])
            nc.vector.scalar_tensor_tensor(
                out=lv1[:, a:b], in0=sa1[:, a:b], scalar=1.0, in1=sa1[:, a:b],
                op0=ALU.subtract, op1=ALU.mult)
            qa, qb = (0, HA - 2) if hi == 0 else (HA - 2, N)
            nc.vector.tensor_mul(out=P12[:, qa:qb], in0=lv1[:, qa + 2:qb + 2],
                                 in1=xp[:, qa + 2:qb + 2])

        # ---------------- chunked: msum(PE) -> G4 -> GL/W -> GEMM -> out --
        G4 = loss.tile([128, N], BF, tag="G4")
        for c in range(NCH):
            lo = c * CW
            for s in range(2):
                cs = lo + s * 512
                q_ps = msq.tile([128, 512], F32, tag="q", name=f"q_{c}_{s}")
                views = [
                    negq[:, :],
                    lv1[:, cs + 2:cs + 514], lv1[:, cs + 3:cs + 515],
                    lv2b[:, cs:cs + 512], lv2b[:, cs + 2:cs + 514],
                    lv3[:, cs:cs + 512], lv3[:, cs + 3:cs + 515],
                ]
                for vi, v in enumerate(views):
                    nc.tensor.matmul(out=q_ps, lhsT=eyen, rhs=v,
                                     start=(vi == 0), stop=(vi == len(views) - 1))
                g32 = ck.tile([128, 512], F32, tag="g32", name=f"g32_{c}_{s}")
                nc.vector.reciprocal_approx_fast(out=g32, in_=q_ps)
                nc.scalar.copy(out=G4[:, cs:cs + 512], in_=g32)

            GL1 = ck.tile([128, CW], BF, tag="GL1", name=f"GL1_{c}")
            GL2 = ck.tile([128, CW], BF, tag="GL2", name=f"GL2_{c}")
            GL3 = ck.tile([128, CW], BF, tag="GL3", name=f"GL3_{c}")
            nc.vector.tensor_mul(out=GL1, in0=lv1[:, lo + 3:lo + 3 + CW],
                                 in1=G4[:, lo:lo + CW])
            nc.vector.tensor_mul(out=GL2, in0=lv2b[:, lo + 2:lo + 2 + CW],
                                 in1=G4[:, lo:lo + CW])
            nc.vector.tensor_mul(out=GL3, in0=lv3[:, lo + 3:lo + 3 + CW],
                                 in1=G4[:, lo:lo + CW])

            W = [ck.tile([128, CW], BF, tag=f"W{k}", name=f"W{k}_{c}")
                 for k in range(KS)]
            nc.vector.tensor_mul(out=W[0], in0=G4[:, lo:lo + CW],
                                 in1=P30[:, lo:lo + CW])
            nc.vector.tensor_mul(out=W[1], in0=G4[:, lo:lo + CW],
                                 in1=P21[:, lo:lo + CW])
            nc.vector.tensor_mul(out=W[2], in0=G4[:, lo:lo + CW],
                                 in1=P12[:, lo:lo + CW])
            nc.vector.tensor_mul(out=W[3], in0=G4[:, lo:lo + CW],
                                 in1=xs1[:, lo + 2:lo + 2 + CW])
            nc.vector.tensor_mul(out=W[4], in0=GL1, in1=xp[:, lo + 4:lo + 4 + CW])
            nc.gpsimd.tensor_mul(out=W[5], in0=GL2, in1=xs1g[:, lo + 4:lo + 4 + CW])
            nc.gpsimd.tensor_mul(out=W[6], in0=GL3, in1=xpg[:, lo + 6:lo + 6 + CW])

            # GEMM: k outer so each fck slice is loaded once for 4 matmuls
            accs = [ppa.tile([128, 512], F32, tag="acc", name=f"acc_{c}_{i}")
                    for i in range(4)]
            for k in range(KS):
                for b in range(2):
                    prow = slice(64 * b, 64 * (b + 1))
                    for sub in range(2):
                        cs = slice(sub * 512, (sub + 1) * 512)
                        nc.tensor.matmul(
                            out=accs[2 * b + sub],
                            lhsT=fck[prow, k, :],
                            rhs=W[k][prow, cs],
                            start=(k == 0), stop=(k == KS - 1),
                        )
            for b in range(2):
                for sub in range(2):
                    i = 2 * b + sub
                    stage = stp.tile([128, 512], BF, tag="stage",
                                     name=f"stage_{c}_{b}_{sub}")
                    nc.scalar.copy(out=stage, in_=accs[i])
                    nc.sync.dma_start(
                        out=out_d[:, b, lo + sub * 512:lo + (sub + 1) * 512],
                        in_=stage)


def build_nc():
    nc = bacc_mod.Bacc(None, target_bir_lowering=False)
    xp_d = nc.dram_tensor("xp", [128, NP], BF, kind="ExternalInput")
    cwdp_d = nc.dram_tensor("cwdp", [32, 128 + ND], F32, kind="ExternalInput")
    cb_d = nc.dram_tensor("cb", [128, 1], F32, kind="ExternalInput")
    fck_d = nc.dram_tensor("fck", [128, KS, 128], BF, kind="ExternalInput")
    eyen_d = nc.dram_tensor("eyen", [128, 128], BF, kind="ExternalInput")
    out_d = nc.dram_tensor("out", [128, 2, N], BF, kind="ExternalOutput")
    with tile.TileContext(nc) as tc:
        kernel_body(tc, xp_d, cwdp_d, cb_d, fck_d, eyen_d, out_d)
    nc.compile()
    return nc


def prep_inputs(deep, x, conv_w, conv_b, fc_w):
    deep = np.asarray(deep, np.float32)
    x = np.asarray(x, np.float32)
    conv_w = np.asarray(conv_w, np.float32)
    conv_b = np.asarray(conv_b, np.float32)
    fc_w = np.asarray(fc_w, np.float32)

    xpad = np.pad(x, ((0, 0), (0, 0), (PAD, PAD)), mode="reflect")
    xp_all = np.ascontiguousarray(xpad.reshape(NCORES, 128, NP)).astype(bf16)
    dp_all = np.ascontiguousarray(deep.reshape(NCORES, 32, ND))
    cw_blk = np.zeros((32, 128), np.float32)
    cw_blk[0:16, 0:64] = conv_w.T
    cw_blk[16:32, 64:128] = conv_w.T
    cb = np.ascontiguousarray(
        np.concatenate([conv_b, conv_b]).reshape(128, 1).astype(np.float32))
    fc3 = fc_w.reshape(128, 64, KS)
    fck_half = np.transpose(fc3, (1, 2, 0)).copy()
    fck_half *= -1.0              # lv is computed negated on-chip
    fck_half[:, PAD, :] *= -0.25  # W_3 = G4*x = 4*(G*x), not lv-scaled
    fck = np.ascontiguousarray(
        np.concatenate([fck_half, fck_half], axis=0)).astype(bf16)
    eyen = np.ascontiguousarray((-np.eye(128, dtype=np.float32)).astype(bf16))
    return [
        {"xp": np.ascontiguousarray(xp_all[ci]),
         "cwdp": np.ascontiguousarray(
             np.concatenate([cw_blk, dp_all[ci]], axis=1)),
         "cb": cb, "fck": fck, "eyen": eyen}
        for ci in range(NCORES)
    ]


def gather_out(results):
    out_full = np.empty((16, 128, N), np.float32)
    for ci in range(NCORES):
        o = results[ci]["out"]
        out_full[2 * ci] = o[:, 0].astype(np.float32)
        out_full[2 * ci + 1] = o[:, 1].astype(np.float32)
    return out_full


_CACHED = {}


def _get_nc():
    if "nc" not in _CACHED:
        _CACHED["nc"] = build_nc()
    return _CACHED["nc"]


def kernel(deep, x, conv_w, conv_b, fc_w):
    in_maps = prep_inputs(deep, x, conv_w, conv_b, fc_w)
    nc = _get_nc()
    res = run_bass_kernel_spmd(nc, in_maps, core_ids=list(range(NCORES)))
    return gather_out(res.results)


# revision 9
# speedup vs baseline: 1.4809x; 1.2662x over previous
"""Trainium2 Bass kernel for nn_DeepConv1d (self-contained).

Math (per batch b):
  xr   = linear-interp(deep, 1024 -> 4096)           # commutes with 1x1 conv
  y    = conv_w @ xr + conv_b                        # == interp(conv_w @ deep + conv_b)
  xs   = GAMA*(y-mean)/(var_unbiased+EPS)            # per-channel over n
  loss_k[c,l] = sech^2(xs_pad[c,l+k]-xs_pad[c,l+3])  # k=0..6, reflect pad 3
  S    = sum_k loss_k ;  W_k = (loss_k/S)*x_pad[:,l+k]
  out[o,l] = sum_{c,k} fc_w[o, 7c+k] * W_k[c,l]

On-chip identities:
  - interp(conv(.)) == conv(interp(.)); interp via first differences D.
  - sum(y^2) computed analytically from ys and D (no pass over y):
      sum y^2 = 4*sum ys^2 + sum ys*(D[j+1]-D[j]) + 0.3125*sum D^2
  - sech^2(d) = 4*sigmoid(2d)*sigmoid(-2d); the normalization scale
    f = GAMA/(var+EPS) folds into the sigmoid's per-partition scale.
  - lv = (sa-1)*sa = -sigmoid'(z), computed in bf16.  Downstream sees
    negated lv; compensated by host-negating fc (k != 3).
  - S/4 = 0.25 + sum sigmoid' accumulated ON THE PE: 7 matmuls with a
    negated-identity lhsT (6 shifted lv views + a -0.25 const tile) into
    PSUM, then one DVE reciprocal_approx_fast gives G4 = 4/S directly.
  - loss_k arrays are shifted views of 3 gap arrays lv_g (g = |k-3|).

Layout: 2 batches per core packed on 128 partitions (64 channels each).
The gap arrays are processed in 4 column-quarters aligned with the four
l-chunks so each chunk's G4/GL/W/GEMM pipeline starts as soon as its
quarter's sigmoids land.  GEMM is issued k-outer so each fck slice loads
once and PE follows the DVE W-mul stream with minimal lag.  GPSIMD is
unused for compute: measured SBUF contention makes concurrent GPSIMD
cost DVE ~2x more than GPSIMD contributes.
"""
import contextlib

import numpy as np
import ml_dtypes

import concourse.bass as bass
import concourse.bacc as bacc_mod
import concourse.mybir as mybir
import concourse.tile as tile
from concourse.bass_utils import run_bass_kernel_spmd

bf16 = ml_dtypes.bfloat16
AF = mybir.ActivationFunctionType
ALU = mybir.AluOpType

KS = 7
PAD = 3
GAMA = 0.5
EPS = 1e-9
N = 4096
ND = 1024
NP = N + 2 * PAD       # 4102
L3 = N + PAD           # 4099: gap array length
NCORES = 8
NCH = 4                # l-chunks
CW = N // NCH          # 1024
SB = [0, 1027, 2051, 3075, 4099]   # gap-array quarter bounds

F32 = mybir.dt.float32
BF = mybir.dt.bfloat16


def kernel_body(tc, xp_d, cwdp_d, cb_d, fck_d, eyen_d, out_d):
    nc = tc.nc

    ctx = contextlib.ExitStack()
    with ctx:
        io = ctx.enter_context(tc.tile_pool(name="io", bufs=1))
        mid = ctx.enter_context(tc.tile_pool(name="mid", bufs=1))
        loss = ctx.enter_context(tc.tile_pool(name="loss", bufs=1))
        sap = ctx.enter_context(tc.tile_pool(name="sap", bufs=3))
        ck = ctx.enter_context(tc.tile_pool(name="ck", bufs=2))
        stp = ctx.enter_context(tc.tile_pool(name="stp", bufs=4))
        pp = ctx.enter_context(tc.tile_pool(name="pp", bufs=1, space="PSUM"))
        msq = ctx.enter_context(tc.tile_pool(name="msq", bufs=2, space="PSUM"))
        ppa = ctx.enter_context(tc.tile_pool(name="ppa", bufs=4, space="PSUM"))

        # ---------------- input DMAs (small first) ----------------
        cwdp = io.tile([32, 128 + ND], F32, tag="cwdp")
        nc.sync.dma_start(out=cwdp, in_=cwdp_d[:, :])
        cb = io.tile([128, 1], F32, tag="cb")
        nc.sync.dma_start(out=cb, in_=cb_d[:, :])
        fck = io.tile([128, KS, 128], BF, tag="fck")
        nc.sync.dma_start(out=fck, in_=fck_d[:, :, :])
        eyen = io.tile([128, 128], BF, tag="eyen")
        nc.sync.dma_start(out=eyen, in_=eyen_d[:, :])
        xp = io.tile([128, NP], BF, tag="xp")          # x reflect-padded
        xs1 = io.tile([128, NP - 1], BF, tag="xs1")    # same, shifted 1 elem
        nc.sync.dma_start(out=xp, in_=xp_d[:, :])
        nc.sync.dma_start(out=xs1, in_=xp_d[:, 1:NP])

        warm = mid.tile([128, 1], F32, tag="warm")
        nc.scalar.activation(out=warm, in_=cb, func=AF.Sigmoid, scale=1.0)
        negq = mid.tile([128, 512], BF, tag="negq")
        nc.vector.memset(negq, -0.25)

        cw = cwdp[:, 0:128]
        dp = cwdp[:, 128:128 + ND]

        # ---------------- conv (PE) + bias (ACT) ------------
        ys_ps = pp.tile([128, ND], F32, tag="ys")
        for h in range(2):
            nc.tensor.matmul(
                out=ys_ps[:, h * 512:(h + 1) * 512],
                lhsT=cw,
                rhs=dp[:, h * 512:(h + 1) * 512],
                start=True, stop=True,
            )
        ysb = mid.tile([128, ND], BF, tag="ysb")
        sumys = mid.tile([128, 1], F32, tag="sumys")
        nc.scalar.activation(out=ysb, in_=ys_ps, func=AF.Identity, bias=cb,
                             scale=1.0, accum_out=sumys)
        # sum((ys+cb)^2) without touching y: ACT Square straight off PSUM
        sy2 = mid.tile([128, 1], F32, tag="sy2")
        dump = mid.tile([128, ND], F32, tag="dump")
        nc.scalar.activation(out=dump, in_=ys_ps, func=AF.Square, bias=cb,
                             scale=1.0, accum_out=sy2)

        # ---------------- stats pieces on DVE (bias cancels in diffs) ----
        Dp = mid.tile([128, ND + 1], BF, tag="Dp")
        nc.vector.memset(Dp[:, 0:1], 0.0)
        nc.vector.memset(Dp[:, ND:ND + 1], 0.0)
        nc.vector.tensor_sub(out=Dp[:, 1:ND], in0=ysb[:, 1:ND],
                             in1=ysb[:, 0:ND - 1])

        # interp coefficient vectors + tiles
        c12 = mid.tile([128, 2], F32, tag="c12")
        nc.vector.memset(c12[:, 0:1], 0.375)
        nc.vector.memset(c12[:, 1:2], 0.125)
        c21 = mid.tile([128, 2], F32, tag="c21")
        nc.vector.memset(c21[:, 0:1], 0.125)
        nc.vector.memset(c21[:, 1:2], 0.375)

        def _rep2(ap_, off, j0, nj):
            return bass.AP(tensor=ap_.tensor, offset=ap_.offset + off + j0,
                           ap=[list(ap_.ap[0]), [1, nj], [0, 2]])

        def _bcast(ap_, nj):
            return bass.AP(tensor=ap_.tensor, offset=ap_.offset,
                           ap=[list(ap_.ap[0]), [0, nj], [1, 2]])

        DD12 = mid.tile([128, ND, 2], BF, tag="DD12")
        DD21 = mid.tile([128, ND, 2], BF, tag="DD21")
        ypad = mid.tile([128, NP], BF, tag="ypad")
        y4 = ypad[:, PAD:PAD + N].rearrange("p (j r) -> p j r", r=4)
        HJ = ND // 2

        def interp_half(h):
            j0 = h * HJ
            nc.vector.tensor_mul(out=DD12[:, j0:j0 + HJ, :],
                                 in0=_rep2(Dp, 0, j0, HJ), in1=_bcast(c12, HJ))
            nc.vector.tensor_mul(out=DD21[:, j0:j0 + HJ, :],
                                 in0=_rep2(Dp, 1, j0, HJ), in1=_bcast(c21, HJ))
            nc.vector.tensor_sub(out=y4[:, j0:j0 + HJ, 0:2],
                                 in0=_rep2(ysb, 0, j0, HJ),
                                 in1=DD12[:, j0:j0 + HJ, :])
            nc.vector.tensor_add(out=y4[:, j0:j0 + HJ, 2:4],
                                 in0=_rep2(ysb, 0, j0, HJ),
                                 in1=DD21[:, j0:j0 + HJ, :])
            if h == 0:
                for i in range(3):  # left reflect: ypad[2-i] = ypad[4+i]
                    nc.vector.tensor_copy(out=ypad[:, 2 - i:3 - i],
                                          in_=ypad[:, 4 + i:5 + i])
            else:
                for i in range(3):  # right reflect
                    nc.vector.tensor_copy(out=ypad[:, N + 3 + i:N + 4 + i],
                                          in_=ypad[:, N + 1 - i:N + 2 - i])

        interp_half(0)

        # remaining stats: sum D^2 (ACT), cross = sum ysb*ddif (DVE STT)
        sd2 = mid.tile([128, 1], F32, tag="sd2")
        dump2 = mid.tile([128, ND + 1], F32, tag="dump2")
        nc.scalar.activation(out=dump2, in_=Dp, func=AF.Square, accum_out=sd2)
        ddif = mid.tile([128, ND], BF, tag="ddif")
        nc.vector.tensor_sub(out=ddif, in0=Dp[:, 1:ND + 1], in1=Dp[:, 0:ND])
        junk = mid.tile([128, ND], BF, tag="junk")
        cross = mid.tile([128, 1], F32, tag="cross")
        nc.vector.scalar_tensor_tensor(
            out=junk, in0=ddif, scalar=1.0, in1=ysb,
            op0=ALU.mult, op1=ALU.mult, accum_out=cross)

        # sum_y = 4*sumys exactly; sum_y2 = 4*sy2 + cross + 0.3125*sd2
        sum_y = mid.tile([128, 1], F32, tag="sum_y")
        nc.vector.tensor_scalar_mul(out=sum_y, in0=sumys, scalar1=4.0)
        e1 = mid.tile([128, 1], F32, tag="e1")
        nc.vector.tensor_scalar_mul(out=e1, in0=sy2, scalar1=4.0)
        e2 = mid.tile([128, 1], F32, tag="e2")
        nc.vector.tensor_scalar(out=e2, in0=sd2, scalar1=0.3125,
                                scalar2=0.0, op0=ALU.mult, op1=ALU.add)
        e3 = mid.tile([128, 1], F32, tag="e3")
        nc.vector.tensor_add(out=e3, in0=e1, in1=e2)
        sum_y2 = mid.tile([128, 1], F32, tag="sum_y2")
        nc.vector.tensor_add(out=sum_y2, in0=e3, in1=cross)
        # mean = sum_y/N; var = (sum_y2 - sum_y*mean)/(N-1); f = GAMA/(var+EPS)
        mean = mid.tile([128, 1], F32, tag="mean")
        nc.vector.tensor_scalar_mul(out=mean, in0=sum_y, scalar1=1.0 / N)
        t0 = mid.tile([128, 1], F32, tag="t0")
        nc.vector.tensor_mul(out=t0, in0=sum_y, in1=mean)
        t2 = mid.tile([128, 1], F32, tag="t2")
        nc.vector.tensor_sub(out=t2, in0=sum_y2, in1=t0)
        denom = mid.tile([128, 1], F32, tag="denom")
        nc.vector.tensor_scalar(out=denom, in0=t2, scalar1=1.0 / (N - 1),
                                scalar2=EPS, op0=ALU.mult, op1=ALU.add)
        inv = mid.tile([128, 1], F32, tag="inv")
        nc.vector.reciprocal(out=inv, in_=denom)
        f2p = mid.tile([128, 1], F32, tag="f2p")
        nc.vector.tensor_scalar_mul(out=f2p, in0=inv, scalar1=2.0 * GAMA)

        # ---------------- gap diffs (bf16 2x), quarters ------------------
        dy1 = loss.tile([128, L3], BF, tag="dy1")
        dy2b = loss.tile([128, L3], BF, tag="dy2b")
        dy3 = loss.tile([128, L3], BF, tag="dy3")
        lv1 = loss.tile([128, L3], BF, tag="lv1")
        lv2b = loss.tile([128, L3], BF, tag="lv2b")
        lv3 = loss.tile([128, L3], BF, tag="lv3")
        P12 = loss.tile([128, N], BF, tag="P12")
        P21 = loss.tile([128, N], BF, tag="P21")
        P30 = loss.tile([128, N], BF, tag="P30")
        G4 = loss.tile([128, N], BF, tag="G4")

        def dyq(q):
            a, b = SB[q], SB[q + 1]
            nc.vector.tensor_sub(out=dy3[:, a:b], in0=ypad[:, 3 + a:3 + b],
                                 in1=ypad[:, a:b])
            nc.vector.tensor_sub(out=dy2b[:, a:b], in0=ypad[:, 3 + a:3 + b],
                                 in1=ypad[:, 1 + a:1 + b])
            nc.vector.tensor_sub(out=dy1[:, a:b], in0=ypad[:, 1 + a:1 + b],
                                 in1=ypad[:, a:b])

        sa_tiles = {}

        def sigq(q):
            a, b = SB[q], SB[q + 1]
            for g, (dy, sa_nm) in enumerate(
                    [(dy3, "sa3"), (dy2b, "sa2"), (dy1, "sa1")]):
                sa = sap.tile([128, 1027], BF, tag=sa_nm, name=f"{sa_nm}_{q}")
                sa_tiles[(sa_nm, q)] = sa
                nc.scalar.activation(out=sa[:, 0:b - a], in_=dy[:, a:b],
                                     func=AF.Sigmoid, scale=f2p)
            return

        def lvpq(q):
            a, b = SB[q], SB[q + 1]
            # P product column ranges per quarter (aligned to x index = l)
            pa, pb = CW * q, CW * (q + 1)
            for g, (lv, P, xin, xoff, poff) in enumerate([
                    (lv3, P30, xp, 0, 0), (lv2b, P21, xs1, 0, 0),
                    (lv1, P12, xp, 2, 2)]):
                sa_nm = ["sa3", "sa2", "sa1"][g]
                sa = sa_tiles[(sa_nm, q)]
                nc.vector.scalar_tensor_tensor(
                    out=lv[:, a:b], in0=sa[:, 0:b - a], scalar=1.0,
                    in1=sa[:, 0:b - a], op0=ALU.subtract, op1=ALU.mult)
                # P written chunk-aligned; lv/x read at +poff/+xoff shifts
                # (SB bounds chosen so lv[pb-1+poff] is inside quarter q)
                nc.vector.tensor_mul(out=P[:, pa:pb],
                                     in0=lv[:, pa + poff:pb + poff],
                                     in1=xin[:, pa + xoff:pb + xoff])

        # per-chunk compute
        def chunk(c):
            lo = c * CW
            for s in range(2):
                cs = lo + s * 512
                q_ps = msq.tile([128, 512], F32, tag="q", name=f"q_{c}_{s}")
                views = [
                    negq[:, :],
                    lv1[:, cs + 2:cs + 514], lv1[:, cs + 3:cs + 515],
                    lv2b[:, cs:cs + 512], lv2b[:, cs + 2:cs + 514],
                    lv3[:, cs:cs + 512], lv3[:, cs + 3:cs + 515],
                ]
                for vi, v in enumerate(views):
                    nc.tensor.matmul(out=q_ps, lhsT=eyen, rhs=v,
                                     start=(vi == 0), stop=(vi == len(views) - 1))
                g32 = ck.tile([128, 512], F32, tag=f"g32{s}",
                              name=f"g32_{c}_{s}")
                nc.vector.reciprocal_approx_fast(out=g32, in_=q_ps)
                nc.scalar.copy(out=G4[:, cs:cs + 512], in_=g32)

            GL1 = ck.tile([128, CW], BF, tag="GL1", name=f"GL1_{c}")
            GL2 = ck.tile([128, CW], BF, tag="GL2", name=f"GL2_{c}")
            GL3 = ck.tile([128, CW], BF, tag="GL3", name=f"GL3_{c}")
            W = [ck.tile([128, CW], BF, tag=f"W{k}", name=f"W{k}_{c}")
                 for k in range(KS)]
            accs = [ppa.tile([128, 512], F32, tag="acc", name=f"acc_{c}_{i}")
                    for i in range(4)]

            def mm(k):
                for b in range(2):
                    prow = slice(64 * b, 64 * (b + 1))
                    for sub in range(2):
                        cs2 = slice(sub * 512, (sub + 1) * 512)
                        nc.tensor.matmul(
                            out=accs[2 * b + sub],
                            lhsT=fck[prow, k, :],
                            rhs=W[k][prow, cs2],
                            start=(k == 0), stop=(k == KS - 1),
                        )

            # W muls in k order; GEMM matmuls follow each W
            nc.vector.tensor_mul(out=W[0], in0=G4[:, lo:lo + CW],
                                 in1=P30[:, lo:lo + CW])
            mm(0)
            nc.vector.tensor_mul(out=W[1], in0=G4[:, lo:lo + CW],
                                 in1=P21[:, lo:lo + CW])
            mm(1)
            nc.vector.tensor_mul(out=W[2], in0=G4[:, lo:lo + CW],
                                 in1=P12[:, lo:lo + CW])
            mm(2)
            nc.vector.tensor_mul(out=W[3], in0=G4[:, lo:lo + CW],
                                 in1=xs1[:, lo + 2:lo + 2 + CW])
            mm(3)
            nc.vector.tensor_mul(out=GL1, in0=lv1[:, lo + 3:lo + 3 + CW],
                                 in1=G4[:, lo:lo + CW])
            nc.vector.tensor_mul(out=W[4], in0=GL1, in1=xp[:, lo + 4:lo + 4 + CW])
            mm(4)
            nc.vector.tensor_mul(out=GL2, in0=lv2b[:, lo + 2:lo + 2 + CW],
                                 in1=G4[:, lo:lo + CW])
            nc.vector.tensor_mul(out=W[5], in0=GL2, in1=xs1[:, lo + 4:lo + 4 + CW])
            mm(5)
            nc.vector.tensor_mul(out=GL3, in0=lv3[:, lo + 3:lo + 3 + CW],
                                 in1=G4[:, lo:lo + CW])
            nc.vector.tensor_mul(out=W[6], in0=GL3, in1=xp[:, lo + 6:lo + 6 + CW])
            mm(6)

            for b in range(2):
                for sub in range(2):
                    i = 2 * b + sub
                    stage = stp.tile([128, 512], BF, tag="stage",
                                     name=f"stage_{c}_{b}_{sub}")
                    nc.scalar.copy(out=stage, in_=accs[i])
                    nc.sync.dma_start(
                        out=out_d[:, b, lo + sub * 512:lo + (sub + 1) * 512],
                        in_=stage)

        # ---------------- pipeline: quarters drive chunks ----------------
        dyq(0)
        sigq(0)
        interp_half(1)
        dyq(1)
        sigq(1)
        lvpq(0)
        dyq(2)
        sigq(2)
        chunk(0)
        lvpq(1)
        dyq(3)
        sigq(3)
        chunk(1)
        lvpq(2)
        chunk(2)
        lvpq(3)
        chunk(3)


def build_nc():
    nc = bacc_mod.Bacc(None, target_bir_lowering=False)
    xp_d = nc.dram_tensor("xp", [128, NP], BF, kind="ExternalInput")
    cwdp_d = nc.dram_tensor("cwdp", [32, 128 + ND], F32, kind="ExternalInput")
    cb_d = nc.dram_tensor("cb", [128, 1], F32, kind="ExternalInput")
    fck_d = nc.dram_tensor("fck", [128, KS, 128], BF, kind="ExternalInput")
    eyen_d = nc.dram_tensor("eyen", [128, 128], BF, kind="ExternalInput")
    out_d = nc.dram_tensor("out", [128, 2, N], BF, kind="ExternalOutput")
    with tile.TileContext(nc) as tc:
        kernel_body(tc, xp_d, cwdp_d, cb_d, fck_d, eyen_d, out_d)
    nc.compile()
    return nc


def prep_inputs(deep, x, conv_w, conv_b, fc_w):
    deep = np.asarray(deep, np.float32)
    x = np.asarray(x, np.float32)
    conv_w = np.asarray(conv_w, np.float32)
    conv_b = np.asarray(conv_b, np.float32)
    fc_w = np.asarray(fc_w, np.float32)

    xpad = np.pad(x, ((0, 0), (0, 0), (PAD, PAD)), mode="reflect")
    xp_all = np.ascontiguousarray(xpad.reshape(NCORES, 128, NP)).astype(bf16)
    dp_all = np.ascontiguousarray(deep.reshape(NCORES, 32, ND))
    cw_blk = np.zeros((32, 128), np.float32)
    cw_blk[0:16, 0:64] = conv_w.T
    cw_blk[16:32, 64:128] = conv_w.T
    cb = np.ascontiguousarray(
        np.concatenate([conv_b, conv_b]).reshape(128, 1).astype(np.float32))
    fc3 = fc_w.reshape(128, 64, KS)
    fck_half = np.transpose(fc3, (1, 2, 0)).copy()
    fck_half *= -1.0              # lv is computed negated on-chip
    fck_half[:, PAD, :] *= -0.25  # W_3 = G4*x = 4*(G*x), not lv-scaled
    fck = np.ascontiguousarray(
        np.concatenate([fck_half, fck_half], axis=0)).astype(bf16)
    eyen = np.ascontiguousarray((-np.eye(128, dtype=np.float32)).astype(bf16))
    return [
        {"xp": np.ascontiguousarray(xp_all[ci]),
         "cwdp": np.ascontiguousarray(
             np.concatenate([cw_blk, dp_all[ci]], axis=1)),
         "cb": cb, "fck": fck, "eyen": eyen}
        for ci in range(NCORES)
    ]


def gather_out(results):
    out_full = np.empty((16, 128, N), np.float32)
    for ci in range(NCORES):
        o = results[ci]["out"]
        out_full[2 * ci] = o[:, 0].astype(np.float32)
        out_full[2 * ci + 1] = o[:, 1].astype(np.float32)
    return out_full


_CACHED = {}


def _get_nc():
    if "nc" not in _CACHED:
        _CACHED["nc"] = build_nc()
    return _CACHED["nc"]


def kernel(deep, x, conv_w, conv_b, fc_w):
    in_maps = prep_inputs(deep, x, conv_w, conv_b, fc_w)
    nc = _get_nc()
    res = run_bass_kernel_spmd(nc, in_maps, core_ids=list(range(NCORES)))
    return gather_out(res.results)
